# revision 1
# baseline (speedup 1.0000x reference)
"""Mean aggregation over sampled neighbors (GNN message passing) on 8 TRN2 cores.

reference:  out[n, :] = mean_j feature[neighbor_idx[n, j], :]
  feature      [200000, 64]  f32
  neighbor_idx [100000, 12]  int
  out          [100000, 64]  f32

Strategy: shard n_nodes across the 8 cores (12500 nodes each); replicate the
feature table into every core's HBM. Each core processes its nodes in tiles
of 128 (one node per SBUF partition). For each tile it issues 12 indirect
DMAs (SWDGE gather, one offset per partition) fetching neighbor j's feature
row for all 128 nodes, accumulates the 12 gathered tiles on the vector
engine, scales by 1/12, and streams the results out.

Note: this runtime exposes no batched-index gather (the extended GPSIMD
dma_gather ucode is unavailable, and indirect DMA consumes exactly one
offset per dest partition), so the gather rate is bound by the SWDGE
per-instruction overhead (~1.5us per 128 rows, measured).
"""

import sys

sys.path.insert(0, "/opt/trn_rl_repo")

import numpy as np

import concourse.bacc as bacc
import concourse.bass as bass
import concourse.tile as tile
from concourse import mybir
from concourse.bass_utils import run_bass_kernel_spmd

P = 128             # SBUF partitions = nodes per tile
N_TOTAL = 200000    # feature table rows
D = 64              # feature dim
N_NODES = 100000
S = 12              # sampled neighbors per node
N_CORES = 8
NODES_PER_CORE = N_NODES // N_CORES          # 12500
N_TILES = -(-NODES_PER_CORE // P)            # 98 node tiles of 128
NODES_PAD = N_TILES * P                      # 12544

_cached = {}


def _build_program():
    nc = bacc.Bacc("TRN2", target_bir_lowering=False)
    feat = nc.dram_tensor("feature", [N_TOTAL, D], mybir.dt.float32,
                          kind="ExternalInput").ap()
    idxt = nc.dram_tensor("idx_t", [P, N_TILES * S], mybir.dt.int32,
                          kind="ExternalInput").ap()
    out = nc.dram_tensor("out", [N_TILES, P, D], mybir.dt.float32,
                         kind="ExternalOutput").ap()

    with tile.TileContext(nc) as tc:
        with tc.tile_pool(name="sbuf", bufs=3) as pool:
            # One DMA for every offset: all later waits on it are satisfied
            # after the first gather, so Tile stops emitting Pool-side waits.
            offs_all = pool.tile([P, N_TILES * S], mybir.dt.int32, tag="offs")
            nc.sync.dma_start(out=offs_all[:], in_=idxt[:])
            for t in range(N_TILES):
                # 12 gathers land in disjoint 64-col slices of ONE tile; a
                # single strided tensor_reduce consumes all of them, so the
                # 12 WAW waits per tile collapse onto one DVE tick.
                g = pool.tile([P, S * D], mybir.dt.float32, tag="g")
                for j in range(S):
                    nc.gpsimd.indirect_dma_start(
                        out=g[:, j * D:(j + 1) * D],
                        out_offset=None,
                        in_=feat[:],
                        in_offset=bass.IndirectOffsetOnAxis(
                            ap=offs_all[:, t * S + j:t * S + j + 1], axis=0),
                    )
                st = pool.tile([P, D], mybir.dt.float32, tag="st")
                # view [P, D, S]: reduce the neighbor axis (stride D) innermost
                nc.vector.tensor_reduce(
                    out=st[:].rearrange("p d -> p d", d=D),
                    in_=g[:].rearrange("p (s d) -> p d s", s=S, d=D),
                    axis=mybir.AxisListType.X,
                    op=mybir.AluOpType.add,
                )
                nc.vector.tensor_scalar_mul(st[:], st[:], 1.0 / S)
                nc.sync.dma_start(out=out[t], in_=st[:])
    nc.compile()
    return nc


def _prep_idx(nbr_shard):
    """[NODES_PER_CORE, S] int -> [P, N_TILES*S] int32 (padded with row 0).

    Layout: [p, t*S + j] = idx of neighbor j of node t*128+p, so the whole
    offsets table loads into SBUF with one contiguous DMA."""
    padded = np.zeros((NODES_PAD, S), dtype=np.int32)
    padded[:NODES_PER_CORE] = nbr_shard
    return np.ascontiguousarray(
        padded.reshape(N_TILES, P, S).transpose(1, 0, 2).reshape(P, N_TILES * S)
    )


def kernel(feature, neighbor_idx, _trace=False, **_run_kwargs):
    feature = np.ascontiguousarray(np.asarray(feature), dtype=np.float32)
    nbr32 = np.asarray(neighbor_idx).astype(np.int32)

    if "nc" not in _cached:
        _cached["nc"] = _build_program()
    nc = _cached["nc"]

    in_maps = [
        {
            "feature": feature,
            "idx_t": _prep_idx(nbr32[c * NODES_PER_CORE:(c + 1) * NODES_PER_CORE]),
        }
        for c in range(N_CORES)
    ]
    res = run_bass_kernel_spmd(
        nc, in_maps, core_ids=list(range(N_CORES)), trace=_trace, **_run_kwargs
    )

    outs = []
    for c in range(N_CORES):
        o = res.results[c]["out"].reshape(NODES_PAD, D)
        outs.append(o[:NODES_PER_CORE])
    full = np.concatenate(outs, axis=0)
    if _trace:
        return full, res
    return full



# revision 5
# speedup vs baseline: 1.6529x; 1.6529x over previous
"""Mean aggregation over sampled neighbors (GNN message passing) on 8 TRN2 cores.

reference:  out[n, :] = mean_j feature[neighbor_idx[n, j], :]
  feature      [200000, 64]  f32
  neighbor_idx [100000, 12]  int
  out          [100000, 64]  f32

Strategy: shard n_nodes across the 8 cores (12500 nodes each, padded to
12544); replicate the feature table in every core's HBM. The old kernel
issued one SWDGE indirect DMA per (node-tile, neighbor) -- 1176 instructions
whose Q7 descriptor generation (~8.6ns/descriptor, engine-serial) took
1.3ms/core. This version uses the extended dma_gather ucode, which packs up
to 1024 row-gathers into ONE instruction (per-engine packet limit 16KB =
64x256B) and, critically, runs on the Q7 cpu pair selected by queue_num --
rotating over 4 SWDGE queues runs 4 descriptor generators in parallel
(~2.6ns/row measured vs 8.6 single-queue).

dma_gather only takes int16 indices (<=32768 rows per window), so the
gather is 2 hops:
  hop1: per (node-window, 32768-row table chunk): gather that window's
        samples falling in the chunk (sorted by row for HBM locality,
        chunk-local int16 idx) -> SBUF -> contiguous writeback to a DRAM
        staging buffer (per-window; <=31360 rows so positions fit int16).
  hop2: per 128-node tile: gather its 1536 samples from the window staging
        with node-major positions -> [128, 12, 64] tile -> strided
        tensor_reduce over the neighbor axis + 1/12 scale -> out rows.
"""

import sys

sys.path.insert(0, "/opt/trn_rl_repo")

import numpy as np

import concourse.bacc as bacc
import concourse.tile as tile
from concourse import mybir
from concourse.bass_utils import run_bass_kernel_spmd

P = 128
N_TOTAL = 200000
D = 64
N_NODES = 100000
S = 12
N_CORES = 8
NODES_PER_CORE = N_NODES // N_CORES          # 12500
NODES_PAD = 12544                            # 98 tiles of 128

CH = 32768                                   # table chunk (int16 idx window)
N_CH = 7                                     # chunks cover 200000 rows
CH_ROWS = [min(CH, N_TOTAL - c * CH) for c in range(N_CH)]

# node windows: staging per window must stay < 32768 rows after padding
W_NODES = [2432] * 5 + [384]                 # sum = 12544
W_SLOTS = [n // P for n in W_NODES]          # 19,19,19,19,19,3
# static per-chunk column capacity (128 rows/col) in the staging layout.
# big window: 29184 samples: chunk mean 4782 (sd 63) -> 40 cols; last chunk
# mean 495 (sd 22) -> 5 cols.  small window: 4608 samples: 755 (sd 25) -> 7
# (chunk 0 also absorbs the 44 pad nodes x 12 zero-idx samples -> 12 cols);
# 78 (sd 9) -> 1.
W_CAPS = [[40] * 6 + [5]] * 5 + [[12] + [7] * 5 + [1]]
W_STAGED = [sum(c) * P for c in W_CAPS]      # 31360 x5, 5504

# sub-gather split: <=8 cols (1024 idxs) per instruction
def _subs(cols):
    out = []
    while cols > 0:
        t = min(8, cols)
        out.append(t)
        cols -= t
    return out

H1_COLS_TOTAL = sum(sum(c) for c in W_CAPS) * P        # 162304 idxs
H2_TOTAL = sum(n * S for n in W_NODES)                 # 150528 idxs
H2_SPLIT = [8, 4]                                      # 12 cols per node tile

_cached = {}


def _build_program():
    nc = bacc.Bacc("TRN2", target_bir_lowering=False, num_swdge_queues=4)
    feat = nc.dram_tensor("feature", [N_TOTAL, D], mybir.dt.float32,
                          kind="ExternalInput").ap()
    h1i = nc.dram_tensor("h1i", [P, H1_COLS_TOTAL // 16], mybir.dt.int16,
                         kind="ExternalInput").ap()
    h2i = nc.dram_tensor("h2i", [P, H2_TOTAL // 16], mybir.dt.int16,
                         kind="ExternalInput").ap()
    stg = [nc.dram_tensor(f"stg{w}", [W_STAGED[w], D], mybir.dt.float32,
                          kind="Internal").ap() for w in range(len(W_NODES))]
    out = nc.dram_tensor("out", [NODES_PAD, D], mybir.dt.float32,
                         kind="ExternalOutput").ap()

    q = [0]

    def rot():
        q[0] = (q[0] + 1) % 4
        return q[0]

    with tile.TileContext(nc) as tc:
        with tc.tile_pool(name="idx", bufs=1) as ipool, \
             tc.tile_pool(name="sbuf", bufs=6) as pool:
            it1 = ipool.tile([P, H1_COLS_TOTAL // 16], mybir.dt.int16, tag="i1")
            nc.sync.dma_start(out=it1[:], in_=h1i[:])
            it2 = ipool.tile([P, H2_TOTAL // 16], mybir.dt.int16, tag="i2")
            nc.sync.dma_start(out=it2[:], in_=h2i[:])

            o1 = 0          # idx offset into hop1 list (elements)
            o2 = 0          # idx offset into hop2 list
            w0 = 0          # node base of window
            for w, nw in enumerate(W_NODES):
                # hop1: gather+writeback per (chunk, sub)
                colbase = 0
                for c in range(N_CH):
                    for sc in _subs(W_CAPS[w][c]):
                        ni = sc * P
                        gt = pool.tile([P, sc, D], mybir.dt.float32, tag="g1")
                        nc.gpsimd.dma_gather(
                            gt[:], feat[c * CH:c * CH + CH_ROWS[c]],
                            it1[:, o1 // 16:(o1 + ni) // 16], ni, ni, D,
                            queue_num=rot())
                        nc.sync.dma_start(
                            out=stg[w][colbase * P:colbase * P + ni].rearrange(
                                "(x p) d -> p x d", p=P, x=sc),
                            in_=gt[:])
                        o1 += ni
                        colbase += sc
                # hop2: per 128-node tile
                for s in range(W_SLOTS[w]):
                    g2 = pool.tile([P, S, D], mybir.dt.float32, tag="g2")
                    cs = 0
                    for hc in H2_SPLIT:
                        ni = hc * P
                        nc.gpsimd.dma_gather(
                            g2[:, cs:cs + hc, :], stg[w][:],
                            it2[:, o2 // 16:(o2 + ni) // 16], ni, ni, D,
                            queue_num=rot())
                        o2 += ni
                        cs += hc
                    st = pool.tile([P, D], mybir.dt.float32, tag="st")
                    nc.vector.tensor_reduce(
                        out=st[:],
                        in_=g2[:].rearrange("p s d -> p d s", s=S, d=D),
                        axis=mybir.AxisListType.X,
                        op=mybir.AluOpType.add)
                    nc.vector.tensor_scalar_mul(st[:], st[:], 1.0 / S)
                    nc.sync.dma_start(
                        out=out[w0 + s * P:w0 + (s + 1) * P], in_=st[:])
                w0 += nw
    nc.compile()
    return nc


def _wrap16(v):
    """flat int16 list -> [128, N/16] SBUF layout (idx k at [k%16, k//16])."""
    v = np.asarray(v, dtype=np.int16)
    w = v.reshape(-1, 16).T.copy()
    return np.ascontiguousarray(np.tile(w, (8, 1)))


def _prep_core(nbr):
    """nbr [NODES_PER_CORE, S] -> (hop1 idx list, hop2 idx list) flat."""
    pad = np.zeros((NODES_PAD, S), dtype=np.int64)
    pad[:NODES_PER_CORE] = nbr
    h1_parts = []
    h2_parts = []
    w0 = 0
    for w, nw in enumerate(W_NODES):
        idx = pad[w0:w0 + nw].reshape(-1)             # node-major q = nu*S+j
        c_of = idx // CH
        order = np.argsort(c_of * (1 << 18) + idx, kind="stable")
        inv = np.empty_like(order)
        inv[order] = np.arange(len(order))
        sidx = idx[order]
        counts = np.bincount(c_of, minlength=N_CH)
        # staging position of sorted sample r: colbase_c*128 + (r - start_c)
        starts = np.concatenate([[0], np.cumsum(counts)])
        colbase = np.concatenate([[0], np.cumsum(W_CAPS[w])])
        staged_sorted = np.empty(len(order), dtype=np.int64)
        for c in range(N_CH):
            m = int(counts[c])
            cap = W_CAPS[w][c] * P
            if m > cap:
                raise RuntimeError(f"chunk overflow w={w} c={c} {m}>{cap}")
            lc = np.zeros(cap, np.int64)
            lc[:m] = sidx[starts[c]:starts[c] + m] - c * CH
            if m > 0:
                lc[m:] = lc[m - 1] if m else 0
            h1_parts.append(lc)
            staged_sorted[starts[c]:starts[c] + m] = colbase[c] * P + np.arange(m)
        staged_q = staged_sorted[inv]                  # q -> staging row
        # hop2 list: k = col*128 + p; col = s*S + j; nu = s*128 + p
        cols = nw // P * S
        col = np.arange(cols)
        s_of, j_of = col // S, col % S
        qk = ((s_of[:, None] * P + np.arange(P)[None, :]) * S + j_of[:, None])
        h2_parts.append(staged_q[qk.reshape(-1)])
        w0 += nw
    h1 = np.concatenate(h1_parts)
    h2 = np.concatenate(h2_parts)
    assert h1.max() < CH and h2.max() < 32768
    return _wrap16(h1), _wrap16(h2)


def kernel(feature, neighbor_idx, _trace=False, **_run_kwargs):
    feature = np.ascontiguousarray(np.asarray(feature), dtype=np.float32)
    nbr = np.asarray(neighbor_idx).astype(np.int64)

    if "nc" not in _cached:
        _cached["nc"] = _build_program()
    nc = _cached["nc"]

    in_maps = []
    for c in range(N_CORES):
        h1, h2 = _prep_core(nbr[c * NODES_PER_CORE:(c + 1) * NODES_PER_CORE])
        in_maps.append({"feature": feature, "h1i": h1, "h2i": h2})
    res = run_bass_kernel_spmd(
        nc, in_maps, core_ids=list(range(N_CORES)), trace=_trace, **_run_kwargs
    )

    outs = [res.results[c]["out"][:NODES_PER_CORE] for c in range(N_CORES)]
    full = np.concatenate(outs, axis=0)
    if _trace:
        return full, res
    return full
